# revision 7
# baseline (speedup 1.0000x reference)
"""DynamicConv (MoE-routed per-sample conv) Trainium2 kernel.

Problem (hardcoded — kernel.py must be self-contained):
  x      (64, 256, 1024, 1) f32
  cond   (64, 256)          f32
  w1     (64, 256)          f32   attention MLP layer 1  (HIDDEN=64, CS=256)
  w2     (4, 64)            f32   attention MLP layer 2  (K=4)
  weight (4, 256, 256, 3, 3) f32  K expert kernels (COUT, CIN, 3, 3)
  bias   (4, 256)           f32

  att    = softmax((relu(cond @ w1.T) @ w2.T) / 30)          (64, 4)
  agg_w  = einsum('bk,koihw->boihw', att, weight)
  agg_b  = att @ bias
  out[b] = conv2d(x[b], agg_w[b], stride 1, pad 1) + agg_b[b] (64, 256, 1024, 1)

Key algebraic facts used:
  * Input width is 1, so with padding (1,1) only the middle column
    (kw == 1) of each 3x3 kernel ever multiplies real data: the conv is a
    1-D conv over L with kernel 3 using weight[..., 1].
  * softmax weights sum to 1, so
      agg = sum_k a_k W_k = W_0 + sum_{k>=1} a_k (W_k - W_0),
    which needs only 3 fused (scale*D_k + acc) vector ops per sample.

Sharding: pure data-parallel over batch. 8 samples per NeuronCore, the
(small) expert kernels / MLP replicated; no cross-core communication.
"""

import os

import numpy as np

import concourse.bass as bass
import concourse.tile as tile
from concourse import bacc, mybir
from concourse.bass_utils import run_bass_kernel_spmd

B, CIN, COUT, CS, K, L = 64, 256, 256, 256, 4, 1024
HIDDEN = CS // 4
TEMPERATURE = 30.0
NCORES = 8
BLOC = B // NCORES  # samples per core

F32 = mybir.dt.float32
# float32r: TF32-like fp32 fast path on the PE (1 cycle/row at free>=256,
# ~11 mantissa bits). Operands must be pre-rounded; x is rounded on host,
# agg weights are rounded by the DVE on write.
F32R = mybir.dt.float32r


def _round_fp32r(a: np.ndarray) -> np.ndarray:
    """Round fp32 to the fp32r grid (round-half-even at 12 low mantissa
    bits) — bit-exact vs walrus fp32_to_fp32r."""
    u = np.ascontiguousarray(a, dtype=np.float32).view(np.uint32).astype(np.uint64)
    bias = 0x7FF + ((u >> 12) & 1)
    return ((u + bias) & np.uint64(0xFFFFF000)).astype(np.uint32).view(np.float32)

# set by kernel() on each call: exec_time_ns of slowest core (needs trace)
LAST_EXEC_TIME_NS = None
TRACE = os.environ.get("BASS_KERNEL_TRACE", "0") == "1"


def _build(nc: bass.Bass):
    """Emit the single-core program (SPMD: every core runs this)."""
    x_d = nc.dram_tensor("x", [BLOC, CIN, L], F32R, kind="ExternalInput").ap()
    condt_d = nc.dram_tensor("condt", [2, 128, BLOC], F32, kind="ExternalInput").ap()
    w1t_d = nc.dram_tensor("w1t", [2, 128, HIDDEN], F32, kind="ExternalInput").ap()
    w2t_d = nc.dram_tensor("w2t", [HIDDEN, K], F32, kind="ExternalInput").ap()
    biask_d = nc.dram_tensor("biask", [K, COUT], F32, kind="ExternalInput").ap()
    # wpack[0] = W0, wpack[k] = W_k - W0 (k=1..3); layout [k][p][i2*768 + kh*256 + o]
    wpack_d = nc.dram_tensor("wpack", [K, 128, 1536], F32, kind="ExternalInput").ap()
    y_d = nc.dram_tensor("y", [BLOC, COUT, L], F32, kind="ExternalOutput").ap()

    from contextlib import ExitStack

    with tile.TileContext(nc) as tc, ExitStack() as ctx:
        wpool = ctx.enter_context(tc.tile_pool(name="wpool", bufs=1))
        rpool = ctx.enter_context(tc.tile_pool(name="rpool", bufs=1))
        rps = ctx.enter_context(tc.tile_pool(name="rps", bufs=2, space="PSUM"))
        xpool = ctx.enter_context(tc.tile_pool(name="xpool", bufs=4))
        aggpool = ctx.enter_context(tc.tile_pool(name="aggpool", bufs=2))
        pspool = ctx.enter_context(tc.tile_pool(name="pspool", bufs=4, space="PSUM"))
        opool = ctx.enter_context(tc.tile_pool(name="opool", bufs=3))

        # ---- resident weights -------------------------------------------
        wt = []
        for k in range(K):
            t = wpool.tile([128, 1536], F32, tag=f"wt{k}")
            nc.sync.dma_start(t[:], wpack_d[k])
            wt.append(t)

        # ---- routing: att = softmax((relu(cond@w1.T)@w2.T)/30) ----------
        condt = rpool.tile([128, 2 * BLOC], F32)
        w1t = rpool.tile([128, 2 * HIDDEN], F32)
        w2t = rpool.tile([HIDDEN, K], F32)
        biask = rpool.tile([K, COUT], F32)
        for i2 in range(2):
            nc.sync.dma_start(condt[:, i2 * BLOC:(i2 + 1) * BLOC], condt_d[i2])
            nc.sync.dma_start(w1t[:, i2 * HIDDEN:(i2 + 1) * HIDDEN], w1t_d[i2])
        nc.sync.dma_start(w2t[:], w2t_d[:])
        nc.sync.dma_start(biask[:], biask_d[:])

        psh = rps.tile([HIDDEN, BLOC], F32, tag="rp")  # hT = w1 @ cond_loc.T
        for i2 in range(2):
            nc.tensor.matmul(
                psh[:],
                lhsT=w1t[:, i2 * HIDDEN:(i2 + 1) * HIDDEN],
                rhs=condt[:, i2 * BLOC:(i2 + 1) * BLOC],
                start=(i2 == 0),
                stop=(i2 == 1),
            )
        ht = rpool.tile([HIDDEN, BLOC], F32)
        nc.scalar.activation(ht[:], psh[:], mybir.ActivationFunctionType.Relu)

        psl = rps.tile([BLOC, K], F32, tag="rp")  # logits (b, k)
        nc.tensor.matmul(psl[:], lhsT=ht[:], rhs=w2t[:])
        e = rpool.tile([BLOC, K], F32)
        nc.scalar.activation(
            e[:], psl[:], mybir.ActivationFunctionType.Exp, scale=1.0 / TEMPERATURE
        )
        ssum = rpool.tile([BLOC, 1], F32)
        nc.vector.tensor_reduce(
            ssum[:], e[:], mybir.AxisListType.X, mybir.AluOpType.add
        )
        rcp = rpool.tile([BLOC, 1], F32)
        nc.vector.reciprocal(rcp[:], ssum[:])
        att = rpool.tile([BLOC, K], F32)
        nc.vector.tensor_scalar(att[:], e[:], rcp[:], None, mybir.AluOpType.mult)

        # att (8p, 4f) -> attrow (1, 32) and attT (4, 8) via a DRAM bounce
        # (partition-crossing SBUF->SBUF DMA trips the sim's conflict checker)
        att_scr = nc.dram_tensor("att_scr", [BLOC, K], F32).ap()
        nc.sync.dma_start(att_scr[:], att[:])
        attrow = rpool.tile([1, BLOC * K], F32)
        nc.sync.dma_start(attrow[:], att_scr.rearrange("b k -> (b k)"))
        attT = rpool.tile([K, BLOC], F32)
        nc.sync.dma_start(attT[:], att_scr.rearrange("b k -> k b"))

        # broadcast att across all 128 partitions: ones(1,128).T @ attrow(1,32)
        ones = rpool.tile([1, 128], F32)
        nc.vector.memset(ones[:], 1.0)
        psbc = rps.tile([128, BLOC * K], F32, tag="rp")
        nc.tensor.matmul(psbc[:], lhsT=ones[:], rhs=attrow[:])
        attbc = rpool.tile([128, BLOC * K], F32)
        nc.scalar.copy(attbc[:], psbc[:])

        # aggregated bias, transposed: aggbT[o, (o2, b)] = sum_k bias[k, o] att[b, k]
        aggbT = rpool.tile([128, 2 * BLOC], F32)
        for o2 in range(2):
            psb = rps.tile([128, BLOC], F32, tag="rp")
            nc.tensor.matmul(
                psb[:], lhsT=biask[:, o2 * 128:(o2 + 1) * 128], rhs=attT[:]
            )
            nc.scalar.copy(aggbT[:, o2 * BLOC:(o2 + 1) * BLOC], psb[:])

        # ---- per-sample: mix weights, conv, bias, store -----------------
        for b in range(BLOC):
            # padded input tiles, one per 128-channel chunk
            xp = []
            for i2 in range(2):
                t = xpool.tile([128, L + 2], F32R, tag=f"xp{i2}")
                nc.gpsimd.memset(t[:, 0:1].bitcast(mybir.dt.uint32), 0)
                nc.gpsimd.memset(t[:, L + 1:L + 2].bitcast(mybir.dt.uint32), 0)
                nc.sync.dma_start(
                    t[:, 1:L + 1], x_d[b, i2 * 128:(i2 + 1) * 128, :]
                )
                xp.append(t)

            # agg = W0 + a1*D1 + a2*D2 + a3*D3   (3 fused DVE ops, in place)
            ag = aggpool.tile([128, 1536], F32R)
            sc = lambda k: attbc[:, b * K + k:b * K + k + 1]
            nc.vector.scalar_tensor_tensor(
                ag[:], wt[1][:], sc(1), wt[0][:],
                mybir.AluOpType.mult, mybir.AluOpType.add,
            )
            nc.vector.scalar_tensor_tensor(
                ag[:], wt[2][:], sc(2), ag[:],
                mybir.AluOpType.mult, mybir.AluOpType.add,
            )
            nc.vector.scalar_tensor_tensor(
                ag[:], wt[3][:], sc(3), ag[:],
                mybir.AluOpType.mult, mybir.AluOpType.add,
            )

            for o2 in range(2):
                ost = opool.tile([128, L], F32, tag="ost")
                for t5 in range(2):  # L tiles of 512
                    ps = pspool.tile([128, 512], F32, tag="ps")
                    n_mm = 0
                    for i2 in range(2):
                        for kh in range(3):
                            nc.tensor.matmul(
                                ps[:],
                                lhsT=ag[
                                    :,
                                    i2 * 768 + kh * 256 + o2 * 128:
                                    i2 * 768 + kh * 256 + o2 * 128 + 128,
                                ],
                                rhs=xp[i2][
                                    :, kh + t5 * 512:kh + t5 * 512 + 512
                                ],
                                start=(n_mm == 0),
                                stop=(n_mm == 5),
                            )
                            n_mm += 1
                    # evict + fused per-(b,o) bias add
                    nc.scalar.activation(
                        ost[:, t5 * 512:(t5 + 1) * 512],
                        ps[:],
                        mybir.ActivationFunctionType.Identity,
                        bias=aggbT[:, o2 * BLOC + b:o2 * BLOC + b + 1],
                        scale=1.0,
                    )
                nc.sync.dma_start(y_d[b, o2 * 128:(o2 + 1) * 128, :], ost[:])

    return nc


def _prep_shared(cond, w1, w2, weight, bias):
    """Host-side layout prep for the replicated tensors."""
    wm = weight[:, :, :, :, 1]  # (K, COUT, CIN, 3) — only kw==1 touches data
    # device layout: [k][p][i2*768 + kh*256 + o], i = i2*128 + p
    wdev = (
        wm.transpose(2, 3, 1, 0)  # (CIN, 3, COUT, K)
        .reshape(2, 128, 3, COUT, K)
        .transpose(4, 1, 0, 2, 3)  # (K, 128, 2, 3, COUT)
        .reshape(K, 128, 1536)
    )
    wpack = wdev.copy()
    wpack[1:] -= wpack[0:1]  # difference trick
    condt = np.ascontiguousarray(cond.T).reshape(2, 128, B)
    w1t = np.ascontiguousarray(w1.T).reshape(2, 128, HIDDEN)
    w2t = np.ascontiguousarray(w2.T)
    return (
        np.ascontiguousarray(wpack),
        condt,
        w1t,
        w2t,
        np.ascontiguousarray(bias),
    )


_CACHED_NC = None


def _get_nc():
    global _CACHED_NC
    if _CACHED_NC is None:
        nc = bacc.Bacc(
            "TRN2",
            target_bir_lowering=False,
            debug=False,
            enable_asserts=True,
            num_devices=NCORES,
        )
        _build(nc)
        nc.compile()
        _CACHED_NC = nc
    return _CACHED_NC


def _make_in_maps(inputs):
    x = np.asarray(inputs["x"], dtype=np.float32)
    cond = np.asarray(inputs["cond"], dtype=np.float32)
    w1 = np.asarray(inputs["w1"], dtype=np.float32)
    w2 = np.asarray(inputs["w2"], dtype=np.float32)
    weight = np.asarray(inputs["weight"], dtype=np.float32)
    bias = np.asarray(inputs["bias"], dtype=np.float32)

    wpack, condt, w1t, w2t, biask = _prep_shared(cond, w1, w2, weight, bias)
    xr = _round_fp32r(x.reshape(B, CIN, L))

    in_maps = []
    for c in range(NCORES):
        sl = slice(c * BLOC, (c + 1) * BLOC)
        in_maps.append(
            {
                "x": np.ascontiguousarray(xr[sl]),
                "condt": np.ascontiguousarray(condt[:, :, sl]),
                "w1t": w1t,
                "w2t": w2t,
                "biask": biask,
                "wpack": wpack,
            }
        )
    return in_maps


def kernel(x, cond, w1, w2, weight, bias):
    global LAST_EXEC_TIME_NS
    in_maps = _make_in_maps(
        {"x": x, "cond": cond, "w1": w1, "w2": w2, "weight": weight, "bias": bias}
    )
    nc = _get_nc()
    res = run_bass_kernel_spmd(
        nc, in_maps, core_ids=list(range(NCORES)), trace=TRACE
    )
    LAST_EXEC_TIME_NS = res.exec_time_ns

    y = np.concatenate([res.results[c]["y"] for c in range(NCORES)], axis=0)
    return y.reshape(B, COUT, L, 1).astype(np.float32)


# revision 8
# speedup vs baseline: 6.0887x; 6.0887x over previous
"""DynamicConv (MoE-routed per-sample conv) Trainium2 kernel.

Problem (hardcoded — kernel.py must be self-contained):
  x      (64, 256, 1024, 1) f32
  cond   (64, 256)          f32
  w1     (64, 256)          f32   attention MLP layer 1  (HIDDEN=64, CS=256)
  w2     (4, 64)            f32   attention MLP layer 2  (K=4)
  weight (4, 256, 256, 3, 3) f32  K expert kernels (COUT, CIN, 3, 3)
  bias   (4, 256)           f32

  att    = softmax((relu(cond @ w1.T) @ w2.T) / 30)          (64, 4)
  agg_w  = einsum('bk,koihw->boihw', att, weight)
  agg_b  = att @ bias
  out[b] = conv2d(x[b], agg_w[b], stride 1, pad 1) + agg_b[b] (64, 256, 1024, 1)

Key algebraic facts used:
  * Input width is 1, so with padding (1,1) only the middle column
    (kw == 1) of each 3x3 kernel ever multiplies real data: the conv is a
    1-D conv over L with kernel 3 using weight[..., 1].
  * softmax weights sum to 1, so
      agg = sum_k a_k W_k = W_0 + sum_{k>=1} a_k (W_k - W_0),
    which needs only 3 fused (scale*D_k + acc) vector ops per sample.

Sharding: pure data-parallel over batch. 8 samples per NeuronCore, the
(small) expert kernels / MLP replicated; no cross-core communication.
"""

import os

import numpy as np

import concourse.bass as bass
import concourse.tile as tile
from concourse import bacc, mybir
from concourse.bass_utils import run_bass_kernel_spmd

B, CIN, COUT, CS, K, L = 64, 256, 256, 256, 4, 1024
HIDDEN = CS // 4
TEMPERATURE = 30.0
NCORES = 8
BLOC = B // NCORES  # samples per core

F32 = mybir.dt.float32
# float32r: TF32-like fp32 fast path on the PE (1 cycle/row at free>=256,
# ~11 mantissa bits). Operands must be pre-rounded; x is rounded on host,
# agg weights are rounded by the DVE on write.
F32R = mybir.dt.float32r


def _round_fp32r(a: np.ndarray) -> np.ndarray:
    """Round fp32 to the fp32r grid (round-half-even at 12 low mantissa
    bits) — bit-exact vs walrus fp32_to_fp32r."""
    u = np.ascontiguousarray(a, dtype=np.float32).view(np.uint32).astype(np.uint64)
    bias = 0x7FF + ((u >> 12) & 1)
    return ((u + bias) & np.uint64(0xFFFFF000)).astype(np.uint32).view(np.float32)


LAST_EXEC_TIME_NS = None
TRACE = os.environ.get("BASS_KERNEL_TRACE", "0") == "1"


def _build(nc: bass.Bass, repeat: int = 1):
    """Emit the single-core program (SPMD: every core runs this).

    repeat > 1 re-emits the whole body N times inside one NEFF — used only
    by the benchmark harness to measure steady-state body time without
    per-execution dispatch overhead."""
    x_d = nc.dram_tensor("x", [BLOC, CIN, L], F32R, kind="ExternalInput").ap()
    condt_d = nc.dram_tensor("condt", [2, 128, BLOC], F32, kind="ExternalInput").ap()
    w1t_d = nc.dram_tensor("w1t", [2, 128, HIDDEN], F32, kind="ExternalInput").ap()
    w2t_d = nc.dram_tensor("w2t", [HIDDEN, K], F32, kind="ExternalInput").ap()
    biask_d = nc.dram_tensor("biask", [K, COUT], F32, kind="ExternalInput").ap()
    # wpack[0] = W0, wpack[k] = W_k - W0 (k=1..3); layout [k][p][i2*768 + kh*256 + o]
    wpack_d = nc.dram_tensor("wpack", [K, 128, 1536], F32, kind="ExternalInput").ap()
    y_d = nc.dram_tensor("y", [BLOC, COUT, L], F32, kind="ExternalOutput").ap()

    from contextlib import ExitStack

    with tile.TileContext(nc) as tc, ExitStack() as ctx:
        pools = dict(
            wpool=ctx.enter_context(tc.tile_pool(name="wpool", bufs=1)),
            rpool=ctx.enter_context(tc.tile_pool(name="rpool", bufs=1)),
            rps=ctx.enter_context(tc.tile_pool(name="rps", bufs=2, space="PSUM")),
            xpool=ctx.enter_context(tc.tile_pool(name="xpool", bufs=4)),
            aggpool=ctx.enter_context(tc.tile_pool(name="aggpool", bufs=2)),
            pspool=ctx.enter_context(tc.tile_pool(name="pspool", bufs=4, space="PSUM")),
            opool=ctx.enter_context(tc.tile_pool(name="opool", bufs=3)),
        )
        dram = dict(
            x_d=x_d, condt_d=condt_d, w1t_d=w1t_d, w2t_d=w2t_d,
            biask_d=biask_d, wpack_d=wpack_d, y_d=y_d,
        )
        for _rep in range(repeat):
            _emit_body(nc, _rep, dram, pools)

    return nc


def _emit_body(nc, _rep, dram, pools):
    x_d, condt_d, w1t_d = dram["x_d"], dram["condt_d"], dram["w1t_d"]
    w2t_d, biask_d, wpack_d, y_d = (
        dram["w2t_d"], dram["biask_d"], dram["wpack_d"], dram["y_d"],
    )
    wpool, rpool, rps, xpool = (
        pools["wpool"], pools["rpool"], pools["rps"], pools["xpool"],
    )
    aggpool, pspool, opool = pools["aggpool"], pools["pspool"], pools["opool"]

    # ---- resident weights -------------------------------------------
    wt = []
    for k in range(K):
        t = wpool.tile([128, 1536], F32, tag=f"wt{k}")
        nc.sync.dma_start(t[:], wpack_d[k])
        wt.append(t)

    # ---- routing: att = softmax((relu(cond@w1.T)@w2.T)/30) ----------
    condt = rpool.tile([128, 2 * BLOC], F32)
    w1t = rpool.tile([128, 2 * HIDDEN], F32)
    w2t = rpool.tile([HIDDEN, K], F32)
    biask = rpool.tile([K, COUT], F32)
    for i2 in range(2):
        nc.sync.dma_start(condt[:, i2 * BLOC:(i2 + 1) * BLOC], condt_d[i2])
        nc.sync.dma_start(w1t[:, i2 * HIDDEN:(i2 + 1) * HIDDEN], w1t_d[i2])
    nc.sync.dma_start(w2t[:], w2t_d[:])
    nc.sync.dma_start(biask[:], biask_d[:])

    psh = rps.tile([HIDDEN, BLOC], F32, tag="rp")  # hT = w1 @ cond_loc.T
    for i2 in range(2):
        nc.tensor.matmul(
            psh[:],
            lhsT=w1t[:, i2 * HIDDEN:(i2 + 1) * HIDDEN],
            rhs=condt[:, i2 * BLOC:(i2 + 1) * BLOC],
            start=(i2 == 0),
            stop=(i2 == 1),
        )
    ht = rpool.tile([HIDDEN, BLOC], F32)
    nc.scalar.activation(ht[:], psh[:], mybir.ActivationFunctionType.Relu)

    psl = rps.tile([BLOC, K], F32, tag="rp")  # logits (b, k)
    nc.tensor.matmul(psl[:], lhsT=ht[:], rhs=w2t[:])
    e = rpool.tile([BLOC, K], F32)
    nc.scalar.activation(
        e[:], psl[:], mybir.ActivationFunctionType.Exp, scale=1.0 / TEMPERATURE
    )
    ssum = rpool.tile([BLOC, 1], F32)
    nc.vector.tensor_reduce(ssum[:], e[:], mybir.AxisListType.X, mybir.AluOpType.add)
    rcp = rpool.tile([BLOC, 1], F32)
    nc.vector.reciprocal(rcp[:], ssum[:])
    att = rpool.tile([BLOC, K], F32)
    nc.vector.tensor_scalar(att[:], e[:], rcp[:], None, mybir.AluOpType.mult)

    # att (8p, 4f) -> attrow (1, 32) and attT (4, 8) via a DRAM bounce
    # (partition-crossing SBUF->SBUF DMA trips the sim's conflict checker)
    att_scr = nc.dram_tensor(f"att_scr{_rep}", [BLOC, K], F32).ap()
    nc.sync.dma_start(att_scr[:], att[:])
    attrow = rpool.tile([1, BLOC * K], F32)
    nc.sync.dma_start(attrow[:], att_scr.rearrange("b k -> (b k)"))
    attT = rpool.tile([K, BLOC], F32)
    nc.sync.dma_start(attT[:], att_scr.rearrange("b k -> k b"))

    # broadcast att across all 128 partitions: ones(1,128).T @ attrow(1,32)
    ones = rpool.tile([1, 128], F32)
    nc.vector.memset(ones[:], 1.0)
    psbc = rps.tile([128, BLOC * K], F32, tag="rp")
    nc.tensor.matmul(psbc[:], lhsT=ones[:], rhs=attrow[:])
    attbc = rpool.tile([128, BLOC * K], F32)
    nc.scalar.copy(attbc[:], psbc[:])

    # aggregated bias, transposed: aggbT[o, (o2, b)] = sum_k bias[k, o] att[b, k]
    aggbT = rpool.tile([128, 2 * BLOC], F32)
    for o2 in range(2):
        psb = rps.tile([128, BLOC], F32, tag="rp")
        nc.tensor.matmul(psb[:], lhsT=biask[:, o2 * 128:(o2 + 1) * 128], rhs=attT[:])
        nc.scalar.copy(aggbT[:, o2 * BLOC:(o2 + 1) * BLOC], psb[:])

    # ---- per-sample: mix weights, conv, bias, store -----------------
    for b in range(BLOC):
        # padded input tiles, one per 128-channel chunk
        xp = []
        for i2 in range(2):
            t = xpool.tile([128, L + 2], F32R, tag=f"xp{i2}")
            nc.gpsimd.memset(t[:, 0:1].bitcast(mybir.dt.uint32), 0)
            nc.gpsimd.memset(t[:, L + 1:L + 2].bitcast(mybir.dt.uint32), 0)
            nc.sync.dma_start(t[:, 1:L + 1], x_d[b, i2 * 128:(i2 + 1) * 128, :])
            xp.append(t)

        # agg = W0 + a1*D1 + a2*D2 + a3*D3   (3 fused DVE ops, in place)
        ag = aggpool.tile([128, 1536], F32R)
        sc = lambda k: attbc[:, b * K + k:b * K + k + 1]
        nc.vector.scalar_tensor_tensor(
            ag[:], wt[1][:], sc(1), wt[0][:],
            mybir.AluOpType.mult, mybir.AluOpType.add,
        )
        nc.vector.scalar_tensor_tensor(
            ag[:], wt[2][:], sc(2), ag[:],
            mybir.AluOpType.mult, mybir.AluOpType.add,
        )
        nc.vector.scalar_tensor_tensor(
            ag[:], wt[3][:], sc(3), ag[:],
            mybir.AluOpType.mult, mybir.AluOpType.add,
        )

        for o2 in range(2):
            ost = opool.tile([128, L], F32, tag="ost")
            for t5 in range(2):  # L tiles of 512
                ps = pspool.tile([128, 512], F32, tag="ps")
                n_mm = 0
                for i2 in range(2):
                    for kh in range(3):
                        nc.tensor.matmul(
                            ps[:],
                            lhsT=ag[
                                :,
                                i2 * 768 + kh * 256 + o2 * 128:
                                i2 * 768 + kh * 256 + o2 * 128 + 128,
                            ],
                            rhs=xp[i2][:, kh + t5 * 512:kh + t5 * 512 + 512],
                            start=(n_mm == 0),
                            stop=(n_mm == 5),
                        )
                        n_mm += 1
                # evict + fused per-(b,o) bias add
                nc.scalar.activation(
                    ost[:, t5 * 512:(t5 + 1) * 512],
                    ps[:],
                    mybir.ActivationFunctionType.Identity,
                    bias=aggbT[:, o2 * BLOC + b:o2 * BLOC + b + 1],
                    scale=1.0,
                )
            nc.sync.dma_start(y_d[b, o2 * 128:(o2 + 1) * 128, :], ost[:])


def _prep_shared(cond, w1, w2, weight, bias):
    """Host-side layout prep for the replicated tensors."""
    wm = weight[:, :, :, :, 1]  # (K, COUT, CIN, 3) — only kw==1 touches data
    # device layout: [k][p][i2*768 + kh*256 + o], i = i2*128 + p
    wdev = (
        wm.transpose(2, 3, 1, 0)  # (CIN, 3, COUT, K)
        .reshape(2, 128, 3, COUT, K)
        .transpose(4, 1, 0, 2, 3)  # (K, 128, 2, 3, COUT)
        .reshape(K, 128, 1536)
    )
    wpack = wdev.copy()
    wpack[1:] -= wpack[0:1]  # difference trick
    condt = np.ascontiguousarray(cond.T).reshape(2, 128, B)
    w1t = np.ascontiguousarray(w1.T).reshape(2, 128, HIDDEN)
    w2t = np.ascontiguousarray(w2.T)
    return (
        np.ascontiguousarray(wpack),
        condt,
        w1t,
        w2t,
        np.ascontiguousarray(bias),
    )


_CACHED_NC = None


def _get_nc():
    global _CACHED_NC
    if _CACHED_NC is None:
        nc = bacc.Bacc(
            "TRN2",
            target_bir_lowering=False,
            debug=False,
            enable_asserts=True,
            num_devices=NCORES,
        )
        _build(nc)
        nc.compile()
        _CACHED_NC = nc
    return _CACHED_NC


def _make_in_maps(inputs):
    x = np.asarray(inputs["x"], dtype=np.float32)
    cond = np.asarray(inputs["cond"], dtype=np.float32)
    w1 = np.asarray(inputs["w1"], dtype=np.float32)
    w2 = np.asarray(inputs["w2"], dtype=np.float32)
    weight = np.asarray(inputs["weight"], dtype=np.float32)
    bias = np.asarray(inputs["bias"], dtype=np.float32)

    wpack, condt, w1t, w2t, biask = _prep_shared(cond, w1, w2, weight, bias)
    xr = _round_fp32r(x.reshape(B, CIN, L))

    in_maps = []
    for c in range(NCORES):
        sl = slice(c * BLOC, (c + 1) * BLOC)
        in_maps.append(
            {
                "x": np.ascontiguousarray(xr[sl]),
                "condt": np.ascontiguousarray(condt[:, :, sl]),
                "w1t": w1t,
                "w2t": w2t,
                "biask": biask,
                "wpack": wpack,
            }
        )
    return in_maps


def kernel(x, cond, w1, w2, weight, bias):
    global LAST_EXEC_TIME_NS
    in_maps = _make_in_maps(
        {"x": x, "cond": cond, "w1": w1, "w2": w2, "weight": weight, "bias": bias}
    )
    nc = _get_nc()
    res = run_bass_kernel_spmd(
        nc, in_maps, core_ids=list(range(NCORES)), trace=TRACE
    )
    LAST_EXEC_TIME_NS = res.exec_time_ns

    y = np.concatenate([res.results[c]["y"] for c in range(NCORES)], axis=0)
    return y.reshape(B, COUT, L, 1).astype(np.float32)


# revision 9
# speedup vs baseline: 6.3408x; 1.0414x over previous
"""DynamicConv (MoE-routed per-sample conv) Trainium2 kernel.

Problem (hardcoded — kernel.py must be self-contained):
  x      (64, 256, 1024, 1) f32
  cond   (64, 256)          f32
  w1     (64, 256)          f32   attention MLP layer 1  (HIDDEN=64, CS=256)
  w2     (4, 64)            f32   attention MLP layer 2  (K=4)
  weight (4, 256, 256, 3, 3) f32  K expert kernels (COUT, CIN, 3, 3)
  bias   (4, 256)           f32

  att    = softmax((relu(cond @ w1.T) @ w2.T) / 30)          (64, 4)
  agg_w  = einsum('bk,koihw->boihw', att, weight)
  agg_b  = att @ bias
  out[b] = conv2d(x[b], agg_w[b], stride 1, pad 1) + agg_b[b] (64, 256, 1024, 1)

Key algebraic facts used:
  * Input width is 1, so with padding (1,1) only the middle column
    (kw == 1) of each 3x3 kernel ever multiplies real data: the conv is a
    1-D conv over L with kernel 3 using weight[..., 1].
  * softmax weights sum to 1, so
      agg = sum_k a_k W_k = W_0 + sum_{k>=1} a_k (W_k - W_0),
    which needs only 3 fused (scale*D_k + acc) vector ops per sample.

Sharding: pure data-parallel over batch. 8 samples per NeuronCore, the
(small) expert kernels / MLP replicated; no cross-core communication.
"""

import os

import numpy as np

import concourse.bass as bass
import concourse.tile as tile
from concourse import bacc, mybir
from concourse.bass_utils import run_bass_kernel_spmd

B, CIN, COUT, CS, K, L = 64, 256, 256, 256, 4, 1024
HIDDEN = CS // 4
TEMPERATURE = 30.0
NCORES = 8
BLOC = B // NCORES  # samples per core

F32 = mybir.dt.float32
# float32r: TF32-like fp32 fast path on the PE (1 cycle/row at free>=256,
# ~11 mantissa bits). Operands must be pre-rounded; x is rounded on host,
# agg weights are rounded by the DVE on write.
F32R = mybir.dt.float32r


def _round_fp32r(a: np.ndarray) -> np.ndarray:
    """Round fp32 to the fp32r grid (round-half-even at 12 low mantissa
    bits) — bit-exact vs walrus fp32_to_fp32r."""
    u = np.ascontiguousarray(a, dtype=np.float32).view(np.uint32).astype(np.uint64)
    bias = 0x7FF + ((u >> 12) & 1)
    return ((u + bias) & np.uint64(0xFFFFF000)).astype(np.uint32).view(np.float32)


LAST_EXEC_TIME_NS = None
TRACE = os.environ.get("BASS_KERNEL_TRACE", "0") == "1"


def _build(nc: bass.Bass, repeat: int = 1):
    """Emit the single-core program (SPMD: every core runs this).

    repeat > 1 re-emits the whole body N times inside one NEFF — used only
    by the benchmark harness to measure steady-state body time without
    per-execution dispatch overhead."""
    x_d = nc.dram_tensor("x", [BLOC, 2, 128, L + 2], F32R, kind="ExternalInput").ap()
    condt_d = nc.dram_tensor("condt", [2, 128, BLOC], F32, kind="ExternalInput").ap()
    w1t_d = nc.dram_tensor("w1t", [2, 128, HIDDEN], F32, kind="ExternalInput").ap()
    w2t_d = nc.dram_tensor("w2t", [HIDDEN, K], F32, kind="ExternalInput").ap()
    biask_d = nc.dram_tensor("biask", [K, COUT], F32, kind="ExternalInput").ap()
    # wpack[0] = W0, wpack[k] = W_k - W0 (k=1..3); layout [k][p][i2*768 + kh*256 + o]
    wpack_d = nc.dram_tensor("wpack", [K, 128, 1536], F32, kind="ExternalInput").ap()
    y_d = nc.dram_tensor("y", [BLOC, COUT, L], F32, kind="ExternalOutput").ap()

    from contextlib import ExitStack

    with tile.TileContext(nc) as tc, ExitStack() as ctx:
        pools = dict(
            wpool=ctx.enter_context(tc.tile_pool(name="wpool", bufs=1)),
            rpool=ctx.enter_context(tc.tile_pool(name="rpool", bufs=1)),
            rps=ctx.enter_context(tc.tile_pool(name="rps", bufs=2, space="PSUM")),
            xpool=ctx.enter_context(tc.tile_pool(name="xpool", bufs=6)),
            aggpool=ctx.enter_context(tc.tile_pool(name="aggpool", bufs=2)),
            pspool=ctx.enter_context(tc.tile_pool(name="pspool", bufs=3, space="PSUM")),
            opool=ctx.enter_context(tc.tile_pool(name="opool", bufs=4)),
        )
        dram = dict(
            x_d=x_d, condt_d=condt_d, w1t_d=w1t_d, w2t_d=w2t_d,
            biask_d=biask_d, wpack_d=wpack_d, y_d=y_d,
        )
        for _rep in range(repeat):
            _emit_body(nc, _rep, dram, pools)

    return nc


def _emit_body(nc, _rep, dram, pools):
    x_d, condt_d, w1t_d = dram["x_d"], dram["condt_d"], dram["w1t_d"]
    w2t_d, biask_d, wpack_d, y_d = (
        dram["w2t_d"], dram["biask_d"], dram["wpack_d"], dram["y_d"],
    )
    wpool, rpool, rps, xpool = (
        pools["wpool"], pools["rpool"], pools["rps"], pools["xpool"],
    )
    aggpool, pspool, opool = pools["aggpool"], pools["pspool"], pools["opool"]

    # ---- resident weights -------------------------------------------
    wt = []
    for k in range(K):
        t = wpool.tile([128, 1536], F32, tag=f"wt{k}")
        nc.sync.dma_start(t[:], wpack_d[k])
        wt.append(t)

    # ---- routing: att = softmax((relu(cond@w1.T)@w2.T)/30) ----------
    condt = rpool.tile([128, 2 * BLOC], F32)
    w1t = rpool.tile([128, 2 * HIDDEN], F32)
    w2t = rpool.tile([HIDDEN, K], F32)
    biask = rpool.tile([K, COUT], F32)
    for i2 in range(2):
        nc.sync.dma_start(condt[:, i2 * BLOC:(i2 + 1) * BLOC], condt_d[i2])
        nc.sync.dma_start(w1t[:, i2 * HIDDEN:(i2 + 1) * HIDDEN], w1t_d[i2])
    nc.sync.dma_start(w2t[:], w2t_d[:])
    nc.sync.dma_start(biask[:], biask_d[:])

    psh = rps.tile([HIDDEN, BLOC], F32, tag="rp")  # hT = w1 @ cond_loc.T
    for i2 in range(2):
        nc.tensor.matmul(
            psh[:],
            lhsT=w1t[:, i2 * HIDDEN:(i2 + 1) * HIDDEN],
            rhs=condt[:, i2 * BLOC:(i2 + 1) * BLOC],
            start=(i2 == 0),
            stop=(i2 == 1),
        )
    ht = rpool.tile([HIDDEN, BLOC], F32)
    nc.scalar.activation(ht[:], psh[:], mybir.ActivationFunctionType.Relu)

    psl = rps.tile([BLOC, K], F32, tag="rp")  # logits (b, k)
    nc.tensor.matmul(psl[:], lhsT=ht[:], rhs=w2t[:])
    e = rpool.tile([BLOC, K], F32)
    nc.scalar.activation(
        e[:], psl[:], mybir.ActivationFunctionType.Exp, scale=1.0 / TEMPERATURE
    )
    ssum = rpool.tile([BLOC, 1], F32)
    nc.vector.tensor_reduce(ssum[:], e[:], mybir.AxisListType.X, mybir.AluOpType.add)
    rcp = rpool.tile([BLOC, 1], F32)
    nc.vector.reciprocal(rcp[:], ssum[:])
    att = rpool.tile([BLOC, K], F32)
    nc.vector.tensor_scalar(att[:], e[:], rcp[:], None, mybir.AluOpType.mult)

    # att (8p, 4f) -> attrow (1, 32) and attT (4, 8) via a DRAM bounce
    # (partition-crossing SBUF->SBUF DMA trips the sim's conflict checker)
    att_scr = nc.dram_tensor(f"att_scr{_rep}", [BLOC, K], F32).ap()
    nc.sync.dma_start(att_scr[:], att[:])
    attrow = rpool.tile([1, BLOC * K], F32)
    nc.sync.dma_start(attrow[:], att_scr.rearrange("b k -> (b k)"))
    attT = rpool.tile([K, BLOC], F32)
    nc.sync.dma_start(attT[:], att_scr.rearrange("b k -> k b"))

    # broadcast att across all 128 partitions: ones(1,128).T @ attrow(1,32)
    ones = rpool.tile([1, 128], F32)
    nc.vector.memset(ones[:], 1.0)
    psbc = rps.tile([128, BLOC * K], F32, tag="rp")
    nc.tensor.matmul(psbc[:], lhsT=ones[:], rhs=attrow[:])
    attbc = rpool.tile([128, BLOC * K], F32)
    nc.scalar.copy(attbc[:], psbc[:])

    # aggregated bias, transposed: aggbT[o, (o2, b)] = sum_k bias[k, o] att[b, k]
    aggbT = rpool.tile([128, 2 * BLOC], F32)
    for o2 in range(2):
        psb = rps.tile([128, BLOC], F32, tag="rp")
        nc.tensor.matmul(psb[:], lhsT=biask[:, o2 * 128:(o2 + 1) * 128], rhs=attT[:])
        nc.scalar.copy(aggbT[:, o2 * BLOC:(o2 + 1) * BLOC], psb[:])

    # ---- per-sample: mix weights, conv, bias, store -----------------
    for b in range(BLOC):
        # padded input tiles, one per 128-channel chunk
        xp = []
        for i2 in range(2):
            t = xpool.tile([128, L + 2], F32R, tag=f"xp{i2}")
            nc.sync.dma_start(t[:], x_d[b, i2])
            xp.append(t)

        # agg = W0 + a1*D1 + a2*D2 + a3*D3   (3 fused DVE ops, in place)
        ag = aggpool.tile([128, 1536], F32R)
        sc = lambda k: attbc[:, b * K + k:b * K + k + 1]
        nc.vector.scalar_tensor_tensor(
            ag[:], wt[1][:], sc(1), wt[0][:],
            mybir.AluOpType.mult, mybir.AluOpType.add,
        )
        nc.vector.scalar_tensor_tensor(
            ag[:], wt[2][:], sc(2), ag[:],
            mybir.AluOpType.mult, mybir.AluOpType.add,
        )
        nc.vector.scalar_tensor_tensor(
            ag[:], wt[3][:], sc(3), ag[:],
            mybir.AluOpType.mult, mybir.AluOpType.add,
        )

        for o2 in range(2):
            ost = opool.tile([128, L], F32, tag="ost")
            ps = pspool.tile([128, L], F32, tag="ps")  # spans 2 PSUM banks
            for t5 in range(2):  # accumulation group per 512-wide bank
                n_mm = 0
                for i2 in range(2):
                    for kh in range(3):
                        nc.tensor.matmul(
                            ps[:, t5 * 512:(t5 + 1) * 512],
                            lhsT=ag[
                                :,
                                i2 * 768 + kh * 256 + o2 * 128:
                                i2 * 768 + kh * 256 + o2 * 128 + 128,
                            ],
                            rhs=xp[i2][:, kh + t5 * 512:kh + t5 * 512 + 512],
                            start=(n_mm == 0),
                            stop=(n_mm == 5),
                        )
                        n_mm += 1
            # evict both banks + fused per-(b,o) bias add
            nc.scalar.activation(
                ost[:],
                ps[:],
                mybir.ActivationFunctionType.Identity,
                bias=aggbT[:, o2 * BLOC + b:o2 * BLOC + b + 1],
                scale=1.0,
            )
            nc.scalar.dma_start(y_d[b, o2 * 128:(o2 + 1) * 128, :], ost[:])


def _prep_shared(cond, w1, w2, weight, bias):
    """Host-side layout prep for the replicated tensors."""
    wm = weight[:, :, :, :, 1]  # (K, COUT, CIN, 3) — only kw==1 touches data
    # device layout: [k][p][i2*768 + kh*256 + o], i = i2*128 + p
    wdev = (
        wm.transpose(2, 3, 1, 0)  # (CIN, 3, COUT, K)
        .reshape(2, 128, 3, COUT, K)
        .transpose(4, 1, 0, 2, 3)  # (K, 128, 2, 3, COUT)
        .reshape(K, 128, 1536)
    )
    wpack = wdev.copy()
    wpack[1:] -= wpack[0:1]  # difference trick
    condt = np.ascontiguousarray(cond.T).reshape(2, 128, B)
    w1t = np.ascontiguousarray(w1.T).reshape(2, 128, HIDDEN)
    w2t = np.ascontiguousarray(w2.T)
    return (
        np.ascontiguousarray(wpack),
        condt,
        w1t,
        w2t,
        np.ascontiguousarray(bias),
    )


_CACHED_NC = None


def _get_nc():
    global _CACHED_NC
    if _CACHED_NC is None:
        nc = bacc.Bacc(
            "TRN2",
            target_bir_lowering=False,
            debug=False,
            enable_asserts=True,
            num_devices=NCORES,
        )
        _build(nc)
        nc.compile()
        _CACHED_NC = nc
    return _CACHED_NC


def _make_in_maps(inputs):
    x = np.asarray(inputs["x"], dtype=np.float32)
    cond = np.asarray(inputs["cond"], dtype=np.float32)
    w1 = np.asarray(inputs["w1"], dtype=np.float32)
    w2 = np.asarray(inputs["w2"], dtype=np.float32)
    weight = np.asarray(inputs["weight"], dtype=np.float32)
    bias = np.asarray(inputs["bias"], dtype=np.float32)

    wpack, condt, w1t, w2t, biask = _prep_shared(cond, w1, w2, weight, bias)
    xr = _round_fp32r(x.reshape(B, CIN, L))
    xpad = np.zeros((B, 2, 128, L + 2), np.float32)
    xpad[:, :, :, 1:L + 1] = xr.reshape(B, 2, 128, L)

    in_maps = []
    for c in range(NCORES):
        sl = slice(c * BLOC, (c + 1) * BLOC)
        in_maps.append(
            {
                "x": np.ascontiguousarray(xpad[sl]),
                "condt": np.ascontiguousarray(condt[:, :, sl]),
                "w1t": w1t,
                "w2t": w2t,
                "biask": biask,
                "wpack": wpack,
            }
        )
    return in_maps


def kernel(x, cond, w1, w2, weight, bias):
    global LAST_EXEC_TIME_NS
    in_maps = _make_in_maps(
        {"x": x, "cond": cond, "w1": w1, "w2": w2, "weight": weight, "bias": bias}
    )
    nc = _get_nc()
    res = run_bass_kernel_spmd(
        nc, in_maps, core_ids=list(range(NCORES)), trace=TRACE
    )
    LAST_EXEC_TIME_NS = res.exec_time_ns

    y = np.concatenate([res.results[c]["y"] for c in range(NCORES)], axis=0)
    return y.reshape(B, COUT, L, 1).astype(np.float32)


# revision 10
# speedup vs baseline: 6.5697x; 1.0361x over previous
"""DynamicConv (MoE-routed per-sample conv) Trainium2 kernel.

Problem (hardcoded — kernel.py must be self-contained):
  x      (64, 256, 1024, 1) f32
  cond   (64, 256)          f32
  w1     (64, 256)          f32   attention MLP layer 1  (HIDDEN=64, CS=256)
  w2     (4, 64)            f32   attention MLP layer 2  (K=4)
  weight (4, 256, 256, 3, 3) f32  K expert kernels (COUT, CIN, 3, 3)
  bias   (4, 256)           f32

  att    = softmax((relu(cond @ w1.T) @ w2.T) / 30)          (64, 4)
  agg_w  = einsum('bk,koihw->boihw', att, weight)
  agg_b  = att @ bias
  out[b] = conv2d(x[b], agg_w[b], stride 1, pad 1) + agg_b[b] (64, 256, 1024, 1)

Key algebraic facts used:
  * Input width is 1, so with padding (1,1) only the middle column
    (kw == 1) of each 3x3 kernel ever multiplies real data: the conv is a
    1-D conv over L with kernel 3 using weight[..., 1].
  * softmax weights sum to 1, so
      agg = sum_k a_k W_k = W_0 + sum_{k>=1} a_k (W_k - W_0),
    which needs only 3 fused (scale*D_k + acc) vector ops per sample.

Sharding: pure data-parallel over batch. 8 samples per NeuronCore, the
(small) expert kernels / MLP replicated; no cross-core communication.
"""

import os

import numpy as np

import concourse.bass as bass
import concourse.tile as tile
from concourse import bacc, mybir
from concourse.bass_utils import run_bass_kernel_spmd

B, CIN, COUT, CS, K, L = 64, 256, 256, 256, 4, 1024
HIDDEN = CS // 4
TEMPERATURE = 30.0
NCORES = 8
BLOC = B // NCORES  # samples per core

F32 = mybir.dt.float32
# float32r: TF32-like fp32 fast path on the PE (1 cycle/row at free>=256,
# ~11 mantissa bits). Operands must be pre-rounded; x is rounded on host,
# agg weights are rounded by the DVE on write.
F32R = mybir.dt.float32r


def _round_fp32r(a: np.ndarray) -> np.ndarray:
    """Round fp32 to the fp32r grid (round-half-even at 12 low mantissa
    bits) — bit-exact vs walrus fp32_to_fp32r."""
    u = np.ascontiguousarray(a, dtype=np.float32).view(np.uint32).astype(np.uint64)
    bias = 0x7FF + ((u >> 12) & 1)
    return ((u + bias) & np.uint64(0xFFFFF000)).astype(np.uint32).view(np.float32)


LAST_EXEC_TIME_NS = None
TRACE = os.environ.get("BASS_KERNEL_TRACE", "0") == "1"


def _build(nc: bass.Bass, repeat: int = 1):
    """Emit the single-core program (SPMD: every core runs this).

    repeat > 1 re-emits the whole body N times inside one NEFF — used only
    by the benchmark harness to measure steady-state body time without
    per-execution dispatch overhead."""
    x_d = nc.dram_tensor("x", [BLOC, 2, 128, L + 2], F32R, kind="ExternalInput").ap()
    condt_d = nc.dram_tensor("condt", [2, 128, BLOC], F32, kind="ExternalInput").ap()
    w1t_d = nc.dram_tensor("w1t", [2, 128, HIDDEN], F32, kind="ExternalInput").ap()
    w2t_d = nc.dram_tensor("w2t", [HIDDEN, K], F32, kind="ExternalInput").ap()
    biask_d = nc.dram_tensor("biask", [K, COUT], F32, kind="ExternalInput").ap()
    # wpack[0] = W0, wpack[k] = W_k - W0 (k=1..3); layout [k][p][i2*768 + kh*256 + o]
    wpack_d = nc.dram_tensor("wpack", [K, 128, 1536], F32, kind="ExternalInput").ap()
    y_d = nc.dram_tensor("y", [BLOC, COUT, L], F32, kind="ExternalOutput").ap()

    from contextlib import ExitStack

    with tile.TileContext(nc) as tc, ExitStack() as ctx:
        pools = dict(
            wpool=ctx.enter_context(tc.tile_pool(name="wpool", bufs=1)),
            rpool=ctx.enter_context(tc.tile_pool(name="rpool", bufs=1)),
            rps=ctx.enter_context(tc.tile_pool(name="rps", bufs=2, space="PSUM")),
            xpool=ctx.enter_context(tc.tile_pool(name="xpool", bufs=6)),
            aggpool=ctx.enter_context(tc.tile_pool(name="aggpool", bufs=2)),
            pspool=ctx.enter_context(tc.tile_pool(name="pspool", bufs=3, space="PSUM")),
            opool=ctx.enter_context(tc.tile_pool(name="opool", bufs=4)),
        )
        dram = dict(
            x_d=x_d, condt_d=condt_d, w1t_d=w1t_d, w2t_d=w2t_d,
            biask_d=biask_d, wpack_d=wpack_d, y_d=y_d,
        )
        for _rep in range(repeat):
            _emit_body(nc, _rep, dram, pools)

    return nc


def _emit_body(nc, _rep, dram, pools):
    x_d, condt_d, w1t_d = dram["x_d"], dram["condt_d"], dram["w1t_d"]
    w2t_d, biask_d, wpack_d, y_d = (
        dram["w2t_d"], dram["biask_d"], dram["wpack_d"], dram["y_d"],
    )
    wpool, rpool, rps, xpool = (
        pools["wpool"], pools["rpool"], pools["rps"], pools["xpool"],
    )
    aggpool, pspool, opool = pools["aggpool"], pools["pspool"], pools["opool"]

    # ---- resident weights -------------------------------------------
    wt = []
    for k in range(K):
        t = wpool.tile([128, 1536], F32, tag=f"wt{k}")
        nc.sync.dma_start(t[:], wpack_d[k])
        wt.append(t)

    # ---- routing: att = softmax((relu(cond@w1.T)@w2.T)/30) ----------
    condt = rpool.tile([128, 2 * BLOC], F32)
    w1t = rpool.tile([128, 2 * HIDDEN], F32)
    w2t = rpool.tile([HIDDEN, K], F32)
    biask = rpool.tile([K, COUT], F32)
    for i2 in range(2):
        nc.sync.dma_start(condt[:, i2 * BLOC:(i2 + 1) * BLOC], condt_d[i2])
        nc.sync.dma_start(w1t[:, i2 * HIDDEN:(i2 + 1) * HIDDEN], w1t_d[i2])
    nc.sync.dma_start(w2t[:], w2t_d[:])
    nc.sync.dma_start(biask[:], biask_d[:])

    psh = rps.tile([HIDDEN, BLOC], F32, tag="rp")  # hT = w1 @ cond_loc.T
    for i2 in range(2):
        nc.tensor.matmul(
            psh[:],
            lhsT=w1t[:, i2 * HIDDEN:(i2 + 1) * HIDDEN],
            rhs=condt[:, i2 * BLOC:(i2 + 1) * BLOC],
            start=(i2 == 0),
            stop=(i2 == 1),
        )
    ht = rpool.tile([HIDDEN, BLOC], F32)
    nc.scalar.activation(ht[:], psh[:], mybir.ActivationFunctionType.Relu)

    psl = rps.tile([BLOC, K], F32, tag="rp")  # logits (b, k)
    nc.tensor.matmul(psl[:], lhsT=ht[:], rhs=w2t[:])
    # stable softmax: e = exp((l - max)/T); bias = -max/T per-partition
    lmax = rpool.tile([BLOC, 1], F32)
    nc.vector.tensor_reduce(lmax[:], psl[:], mybir.AxisListType.X, mybir.AluOpType.max)
    nmax = rpool.tile([BLOC, 1], F32)
    nc.scalar.mul(nmax[:], lmax[:], -1.0 / TEMPERATURE)
    e = rpool.tile([BLOC, K], F32)
    nc.scalar.activation(
        e[:], psl[:], mybir.ActivationFunctionType.Exp,
        bias=nmax[:], scale=1.0 / TEMPERATURE,
    )
    ssum = rpool.tile([BLOC, 1], F32)
    nc.vector.tensor_reduce(ssum[:], e[:], mybir.AxisListType.X, mybir.AluOpType.add)
    rcp = rpool.tile([BLOC, 1], F32)
    nc.vector.reciprocal(rcp[:], ssum[:])
    att = rpool.tile([BLOC, K], F32)
    nc.vector.tensor_scalar(att[:], e[:], rcp[:], None, mybir.AluOpType.mult)

    # att (8p, 4f) -> attrow (1, 32) and attT (4, 8) via a DRAM bounce
    # (partition-crossing SBUF->SBUF DMA trips the sim's conflict checker)
    att_scr = nc.dram_tensor(f"att_scr{_rep}", [BLOC, K], F32).ap()
    nc.sync.dma_start(att_scr[:], att[:])
    attrow = rpool.tile([1, BLOC * K], F32)
    nc.sync.dma_start(attrow[:], att_scr.rearrange("b k -> (b k)"))
    attT = rpool.tile([K, BLOC], F32)
    nc.sync.dma_start(attT[:], att_scr.rearrange("b k -> k b"))

    # broadcast att across all 128 partitions: ones(1,128).T @ attrow(1,32)
    ones = rpool.tile([1, 128], F32)
    nc.vector.memset(ones[:], 1.0)
    psbc = rps.tile([128, BLOC * K], F32, tag="rp")
    nc.tensor.matmul(psbc[:], lhsT=ones[:], rhs=attrow[:])
    attbc = rpool.tile([128, BLOC * K], F32)
    nc.scalar.copy(attbc[:], psbc[:])

    # aggregated bias, transposed: aggbT[o, (o2, b)] = sum_k bias[k, o] att[b, k]
    aggbT = rpool.tile([128, 2 * BLOC], F32)
    for o2 in range(2):
        psb = rps.tile([128, BLOC], F32, tag="rp")
        nc.tensor.matmul(psb[:], lhsT=biask[:, o2 * 128:(o2 + 1) * 128], rhs=attT[:])
        nc.scalar.copy(aggbT[:, o2 * BLOC:(o2 + 1) * BLOC], psb[:])

    # ---- per-sample: mix weights, conv, bias, store -----------------
    for b in range(BLOC):
        # padded input tiles, one per 128-channel chunk
        xp = []
        for i2 in range(2):
            t = xpool.tile([128, L + 2], F32R, tag=f"xp{i2}")
            nc.sync.dma_start(t[:], x_d[b, i2])
            xp.append(t)

        # agg = W0 + a1*D1 + a2*D2 + a3*D3   (3 fused DVE ops, in place)
        ag = aggpool.tile([128, 1536], F32R)
        sc = lambda k: attbc[:, b * K + k:b * K + k + 1]
        nc.vector.scalar_tensor_tensor(
            ag[:], wt[1][:], sc(1), wt[0][:],
            mybir.AluOpType.mult, mybir.AluOpType.add,
        )
        nc.vector.scalar_tensor_tensor(
            ag[:], wt[2][:], sc(2), ag[:],
            mybir.AluOpType.mult, mybir.AluOpType.add,
        )
        nc.vector.scalar_tensor_tensor(
            ag[:], wt[3][:], sc(3), ag[:],
            mybir.AluOpType.mult, mybir.AluOpType.add,
        )

        for o2 in range(2):
            ost = opool.tile([128, L], F32, tag="ost")
            ps = pspool.tile([128, L], F32, tag="ps")  # spans 2 PSUM banks
            for t5 in range(2):  # accumulation group per 512-wide bank
                n_mm = 0
                for i2 in range(2):
                    for kh in range(3):
                        nc.tensor.matmul(
                            ps[:, t5 * 512:(t5 + 1) * 512],
                            lhsT=ag[
                                :,
                                i2 * 768 + kh * 256 + o2 * 128:
                                i2 * 768 + kh * 256 + o2 * 128 + 128,
                            ],
                            rhs=xp[i2][:, kh + t5 * 512:kh + t5 * 512 + 512],
                            start=(n_mm == 0),
                            stop=(n_mm == 5),
                        )
                        n_mm += 1
            # evict both banks + fused per-(b,o) bias add
            nc.scalar.activation(
                ost[:],
                ps[:],
                mybir.ActivationFunctionType.Identity,
                bias=aggbT[:, o2 * BLOC + b:o2 * BLOC + b + 1],
                scale=1.0,
            )
            nc.scalar.dma_start(y_d[b, o2 * 128:(o2 + 1) * 128, :], ost[:])


def _prep_shared(cond, w1, w2, weight, bias):
    """Host-side layout prep for the replicated tensors."""
    wm = weight[:, :, :, :, 1]  # (K, COUT, CIN, 3) — only kw==1 touches data
    # device layout: [k][p][i2*768 + kh*256 + o], i = i2*128 + p
    wdev = (
        wm.transpose(2, 3, 1, 0)  # (CIN, 3, COUT, K)
        .reshape(2, 128, 3, COUT, K)
        .transpose(4, 1, 0, 2, 3)  # (K, 128, 2, 3, COUT)
        .reshape(K, 128, 1536)
    )
    wpack = wdev.copy()
    wpack[1:] -= wpack[0:1]  # difference trick
    condt = np.ascontiguousarray(cond.T).reshape(2, 128, B)
    w1t = np.ascontiguousarray(w1.T).reshape(2, 128, HIDDEN)
    w2t = np.ascontiguousarray(w2.T)
    return (
        np.ascontiguousarray(wpack),
        condt,
        w1t,
        w2t,
        np.ascontiguousarray(bias),
    )


_CACHED_NC = None


def _get_nc():
    global _CACHED_NC
    if _CACHED_NC is None:
        nc = bacc.Bacc(
            "TRN2",
            target_bir_lowering=False,
            debug=False,
            enable_asserts=True,
            num_devices=NCORES,
        )
        _build(nc)
        nc.compile()
        _CACHED_NC = nc
    return _CACHED_NC


def _make_in_maps(inputs):
    x = np.asarray(inputs["x"], dtype=np.float32)
    cond = np.asarray(inputs["cond"], dtype=np.float32)
    w1 = np.asarray(inputs["w1"], dtype=np.float32)
    w2 = np.asarray(inputs["w2"], dtype=np.float32)
    weight = np.asarray(inputs["weight"], dtype=np.float32)
    bias = np.asarray(inputs["bias"], dtype=np.float32)

    wpack, condt, w1t, w2t, biask = _prep_shared(cond, w1, w2, weight, bias)
    xr = _round_fp32r(x.reshape(B, CIN, L))
    xpad = np.zeros((B, 2, 128, L + 2), np.float32)
    xpad[:, :, :, 1:L + 1] = xr.reshape(B, 2, 128, L)

    in_maps = []
    for c in range(NCORES):
        sl = slice(c * BLOC, (c + 1) * BLOC)
        in_maps.append(
            {
                "x": np.ascontiguousarray(xpad[sl]),
                "condt": np.ascontiguousarray(condt[:, :, sl]),
                "w1t": w1t,
                "w2t": w2t,
                "biask": biask,
                "wpack": wpack,
            }
        )
    return in_maps


def kernel(x, cond, w1, w2, weight, bias):
    global LAST_EXEC_TIME_NS
    in_maps = _make_in_maps(
        {"x": x, "cond": cond, "w1": w1, "w2": w2, "weight": weight, "bias": bias}
    )
    nc = _get_nc()
    res = run_bass_kernel_spmd(
        nc, in_maps, core_ids=list(range(NCORES)), trace=TRACE
    )
    LAST_EXEC_TIME_NS = res.exec_time_ns

    y = np.concatenate([res.results[c]["y"] for c in range(NCORES)], axis=0)
    return y.reshape(B, COUT, L, 1).astype(np.float32)
